# revision 33
# baseline (speedup 1.0000x reference)
"""Trainium2 Bass kernel for nn_ChaosTransformer_22333829939822.

Key mathematical reduction (verified against the reference):
the torch-style ``view(B, H, L, E//H)`` on a [B, L, E] tensor is a raw
row-major reshape, which makes head h attend only within the 256-position
block [h*256, (h+1)*256).  The output ``dec[:, -96:, 0]`` therefore depends
only on the last 256 positions of each batch.  Each core runs one batch's
[256, 256] residual-stream transformer; attention operates on the
[2048, 32] head-view of the 256x256 block.

Sharding: data-parallel over batch B across 4 of the 8 cores (one batch
per core, fully independent, no collectives).

Layouts on device:
- residual stream X kept position-major ([pos, ch], for LayerNorm) and
  channel-major XT ([ch, pos], matmul operand).
- attention in bf16.  Scores are built KEY-major: ST_c[s, q] so that the
  exp'd tiles feed A@V directly as the moving operand with keys on the
  contraction (partition) axis.  Row sums for the softmax denominator come
  from ones-column matmuls; normalization happens once at the end (scores
  are provably tiny here: |SCALE*decay*S| < ~3, so exp needs no max shift).
- the query c-block axis lives on SBUF partitions; 4x row-packed K=32
  matmuls need the Q tile at 4 rotations of its 32-partition blocks,
  produced by permutation matmuls (host supplies the 0/1 matrices).

Perf notes (vs the first working version):
- ALL constants arrive in 4 contiguous [128, C] packs (one DMA each,
  multi-KB descriptors) instead of ~98 small dma_starts; LN gain/bias
  rows are pre-broadcast host-side.
- the decay tile is built by K=1 broadcast MATMULS from a c-major
  time-diff row (the old strided partition_broadcast expanded to ~65k
  4-byte DMA descriptors and stalled the kernel ~45us).
- softmax row-sum normalization: one reciprocal_approx_fast over the
  whole [128,2,qw] tile + broadcast matmuls (the old path did 16
  single-partition reciprocals plus a DRAM round-trip per layer).
- the output leaves as a [1, 96] row (1 descriptor) instead of [96, 1].
- every 3rd softmax-exp tile is computed on the DVE with a Schraudolph
  bitcast exp (the ACT engine is otherwise the attention-phase
  bottleneck alongside the PE; measured engines all ~80-95% busy there).
"""

import sys
import numpy as np

sys.path.insert(0, "/opt/trn_rl_repo")

import concourse.bass as bass
import concourse.tile as tile
from concourse import mybir
from concourse.masks import make_identity

F32 = mybir.dt.float32
BF16 = mybir.dt.bfloat16
I32 = mybir.dt.int32
# Schraudolph fast-exp constants: exp(x) ~= bitcast_f32(int(SCH_A*x + SCH_B)),
# max rel err ~3% on |x|<3 — same scale as fp8 A-quantization, which costs
# only ~2e-4 on the final output (numpy-simulated).  Used to offload a third
# of the softmax exps from the saturated ACT engine onto the DVE.
SCH_A = float(2 ** 23 / np.log(2))
SCH_B = float(127 * 2 ** 23 - 366393)
ADD = mybir.AluOpType.add
SUB = mybir.AluOpType.subtract
MULT = mybir.AluOpType.mult
MAX = mybir.AluOpType.max
ASR = mybir.AluOpType.arith_shift_right
AF = mybir.ActivationFunctionType

B, L, D, E, DFF, LYR, PRED = 4, 2048, 7, 256, 1024, 2, 96
FACTOR = 5.0
SCALE = 1.0 / float(np.sqrt(FACTOR))
LN_SCALE = float(np.log(SCALE))
EPS = 1e-5
P0 = L - 256          # 1792: start of the last 256-position block
QLO2 = 128            # layer-2 computes query positions [128, 256)
NPOS = 256

# ---- packed-constant column layouts (host and device share these) ----
# PF0 (fp32, urgent): td (c-major row) / xT / Wemb / small biases / proj
TD8R = 0                      # [1, 2048] td8row[0, c*256+q] = td[8q+c]
XT_O = 2048                   # [7, 256]
WEMB_O = XT_O + 256           # [7, 256]
BEMBPP_O = WEMB_O + 256       # [128, 2]
BEMBR_O = BEMBPP_O + 2        # [1, 256]
BQ_O = BEMBR_O + 256          # [128, 2] per layer
BK_O = BQ_O + 2 * LYR         # [128, 2] per layer
B1_O = BK_O + 2 * LYR         # [128, 8] per layer
WP_O = B1_O + 8 * LYR         # [128, 2]
BPROJ_O = WP_O + 2            # [1, 1]
CF0 = BPROJ_O + 1

# PF1 (fp32): 10 pre-broadcast LN tiles [128, 256] each
# order: ln1g0 ln1b0 ln2g0 ln2b0 ln1g1 ln1b1 ln2g1 ln2b1 lnfg lnfb
CF1 = 10 * 256

# PB0 (bf16, urgent): Wq/Wk/Wv [(l,k)->128x256], Prot, bias rows
WQ_O = 0                      # 4 x 256
WK_O = WQ_O + 4 * 256
WV_O = WK_O + 4 * 256
PROT_O = WV_O + 4 * 256       # 3 x 128
BROW_O = PROT_O + 3 * 128     # 6 x [1, 256] (row 0): bv0 bv1 bo0 bo1 b20 b21
CB0 = BROW_O + 6 * 256

# PB1 (bf16): Wo [(l,h)], W1 [(l,k)->128x1024], W2 [(l,dk)->128x256]
WO_O = 0                      # 4 x 256
W1_O = WO_O + 4 * 256         # 4 x 1024
W2_O = W1_O + 4 * 1024        # 16 x 256
CB1 = W2_O + 16 * 256


def chaos_kernel(tc, outs, ins):
    import contextlib

    nc = tc.nc
    with contextlib.ExitStack() as ctx:
        _chaos_body(tc, nc, ctx, outs, ins)


def _chaos_body(tc, nc, ctx, outs, ins):
    WDT = BF16
    const = ctx.enter_context(tc.tile_pool(name="const", bufs=1))
    work = ctx.enter_context(tc.tile_pool(name="work", bufs=3))
    atp = ctx.enter_context(tc.tile_pool(name="atp", bufs=24))
    psw = ctx.enter_context(tc.tile_pool(name="psw", bufs=3, space="PSUM"))
    psacc = ctx.enter_context(tc.tile_pool(name="psacc", bufs=1, space="PSUM"))

    dma = nc.sync.dma_start

    def seed_bias(ps_ap, brow_ap, m, n):
        """PSUM <- bias row broadcast over m partitions (K=1 matmul)."""
        ones = ones_row if brow_ap.dtype == F32 else ones_row_w
        nc.tensor.matmul(
            ps_ap, ones[0:1, :m], brow_ap,
            start=True, stop=False,
        )

    def layernorm(x_ap, rows, g_b, b_b, out_ap):
        st = work.tile([128, 6], F32, tag="bn_st")
        nc.vector.bn_stats(st[:rows], x_ap)
        mv = work.tile([128, 2], F32, tag="bn_mv")
        nc.vector.bn_aggr(mv[:rows], st[:rows])
        sd = work.tile([128, 1], F32, tag="bn_sd")
        nc.scalar.activation(sd[:rows], mv[:rows, 1:2], AF.Sqrt,
                             bias=eps_t[:rows])
        nc.vector.reciprocal(sd[:rows], sd[:rows])
        t = work.tile([128, NPOS], F32, tag="ln_t")
        nc.vector.tensor_scalar(t[:rows], x_ap, mv[:rows, 0:1], sd[:rows],
                                SUB, MULT)
        nc.vector.tensor_mul(t[:rows], t[:rows], g_b[:rows])
        nc.vector.tensor_add(out_ap, t[:rows], b_b[:rows])

    # ---------------- packed constant loads (urgency order) ----------------
    PF0t = const.tile([128, CF0], F32, tag="PF0")
    dma(out=PF0t[:], in_=ins["PF0"][:])
    PB0t = const.tile([128, CB0], WDT, tag="PB0")
    dma(out=PB0t[:], in_=ins["PB0"][:])
    PF1t = const.tile([128, CF1], F32, tag="PF1")
    dma(out=PF1t[:], in_=ins["PF1"][:])
    PB1t = const.tile([128, CB1], WDT, tag="PB1")
    dma(out=PB1t[:], in_=ins["PB1"][:])

    xT_sb = PF0t[0:7, XT_O:XT_O + 256]
    Wemb_sb = PF0t[0:7, WEMB_O:WEMB_O + 256]
    bemb_pp = PF0t[:, BEMBPP_O:BEMBPP_O + 2]
    bemb_r = PF0t[0:1, BEMBR_O:BEMBR_O + 256]
    bq_t = {l: PF0t[:, BQ_O + 2 * l:BQ_O + 2 * l + 2] for l in range(LYR)}
    bk_t = {l: PF0t[:, BK_O + 2 * l:BK_O + 2 * l + 2] for l in range(LYR)}
    b1_t = {l: PF0t[:, B1_O + 8 * l:B1_O + 8 * l + 8] for l in range(LYR)}
    Wp_sb = PF0t[:, WP_O:WP_O + 2]
    bprow = PF0t[0:1, BPROJ_O:BPROJ_O + 1]

    ln_names = ["ln1g0", "ln1b0", "ln2g0", "ln2b0",
                "ln1g1", "ln1b1", "ln2g1", "ln2b1", "lnfg", "lnfb"]
    ln_b = {}
    for i, nm in enumerate(ln_names):
        v = PF1t[:, 256 * i:256 * (i + 1)]
        if nm in ("lnfg", "lnfb"):
            ln_b[nm] = v
        else:
            ln_b[(nm[:4], int(nm[4]))] = v

    Wq_t, Wk_t, Wv_t, Wo_t, W1_t, W2_t = {}, {}, {}, {}, {}, {}
    for l in range(LYR):
        for k in range(2):
            i = 2 * l + k
            Wq_t[(l, k)] = PB0t[:, WQ_O + 256 * i:WQ_O + 256 * (i + 1)]
            Wk_t[(l, k)] = PB0t[:, WK_O + 256 * i:WK_O + 256 * (i + 1)]
            Wv_t[(l, k)] = PB0t[:, WV_O + 256 * i:WV_O + 256 * (i + 1)]
            W1_t[(l, k)] = PB1t[:, W1_O + 1024 * i:W1_O + 1024 * (i + 1)]
        for h in range(2):
            i = 2 * l + h
            Wo_t[(l, h)] = PB1t[:, WO_O + 256 * i:WO_O + 256 * (i + 1)]
        for dk in range(8):
            i = 8 * l + dk
            W2_t[(l, dk)] = PB1t[:, W2_O + 256 * i:W2_O + 256 * (i + 1)]
    Prot_t = {r: PB0t[:, PROT_O + 128 * r:PROT_O + 128 * (r + 1)]
              for r in range(3)}
    brows = {}
    for i, (nm, l) in enumerate([("bv", 0), ("bv", 1), ("bo", 0),
                                 ("bo", 1), ("b2", 0), ("b2", 1)]):
        brows[(nm, l)] = PB0t[0:1, BROW_O + 256 * i:BROW_O + 256 * (i + 1)]

    ident = const.tile([128, 128], F32, tag="ident")
    make_identity(nc, ident[:])
    ones_col = const.tile([128, 1], BF16, tag="ones_col")
    nc.vector.memset(ones_col[:], 1.0)
    ones_row = const.tile([1, 128], F32, tag="ones_row")
    nc.vector.memset(ones_row[:], 1.0)
    ones_row_w = const.tile([1, 128], WDT, tag="ones_row_w")
    nc.vector.memset(ones_row_w[:], 1.0)
    eps_t = const.tile([128, 1], F32, tag="eps")
    nc.vector.memset(eps_t[:], EPS)
    ones32 = const.tile([128, 32], F32, tag="ones32")
    nc.vector.memset(ones32[:], 1.0)
    lnsc_t = const.tile([128, 1], F32, tag="lnsc")
    nc.vector.memset(lnsc_t[:], LN_SCALE)

    # ---------------- decay tiles via K=1 broadcast matmuls ----------------
    # D_t[h][32j+d, q] = SCALE * exp(-td[q*8 + (4h+j)]/F)
    #                  = exp(-td8row[(4h+j)*256 + q]/F + ln SCALE)
    T_ps = psw.tile([128, 2, 512], F32, tag="qk")
    nc.vector.memset(T_ps[:, :, :NPOS], 0.0)
    for h in range(2):
        for j in range(4):
            c = 4 * h + j
            nc.tensor.matmul(
                T_ps[32 * j:32 * (j + 1), h, :NPOS],
                ones_row[0:1, :32],
                PF0t[0:1, TD8R + c * NPOS:TD8R + (c + 1) * NPOS],
                start=False, stop=False, skip_group_check=True,
                tile_position=(0, 32 * j))
    D_t = {}
    for h in range(2):
        t = const.tile([128, NPOS], F32, tag=f"D{h}")
        nc.scalar.activation(t[:], T_ps[:, h, :NPOS], AF.Exp,
                             bias=lnsc_t[:], scale=-1.0 / FACTOR)
        D_t[h] = t

    # ---------------- embedding ----------------
    X_t, XT_t = {}, {}
    for p in range(2):  # position-major X
        ps = psw.tile([128, 1024], F32, tag="qk")
        seed_bias(ps[:, :E], bemb_r, 128, E)
        nc.tensor.matmul(ps[:, :E],
                         xT_sb[:, p * 128:(p + 1) * 128],
                         Wemb_sb[:], start=False, stop=True)
        t = const.tile([128, NPOS], F32, tag=f"X{p}")
        nc.vector.tensor_copy(t[:], ps[:, :E])
        X_t[p] = t
    for k in range(2):  # channel-major XT
        ps = psw.tile([128, 1024], F32, tag="qk")
        nc.tensor.matmul(ps[:, :NPOS],
                         Wemb_sb[:, k * 128:(k + 1) * 128],
                         xT_sb[:], start=True, stop=True)
        t = const.tile([128, NPOS], WDT, tag=f"XT{k}")
        nc.vector.tensor_scalar_add(t[:], ps[:, :NPOS], bemb_pp[:, k:k + 1])
        XT_t[k] = t

    # ---------------- transformer layers ----------------
    for l in range(LYR):
        qlo, qhi = (0, NPOS) if l == 0 else (QLO2, NPOS)
        qw = qhi - qlo
        pos_chunks = ([(0, 0, 128), (1, 0, 128)] if l == 0
                      else [(1, 0, 128)])
        # (X-tile index, row offset within tile, nrows) for output positions

        # ---- K projection -> KT channel-major bf16 [128, 256] x2
        KT = {}
        for Jt in range(2):
            ps = psw.tile([128, 1024], F32, tag="qk")
            for k in range(2):
                nc.tensor.matmul(
                    ps[:, :NPOS],
                    Wk_t[(l, k)][:, Jt * 128:(Jt + 1) * 128],
                    XT_t[k][:],
                    start=(k == 0), stop=(k == 1))
            t = work.tile([128, NPOS], BF16, tag=f"KT{Jt}")
            nc.vector.tensor_scalar_add(t[:], ps[:, :NPOS], bk_t[l][:, Jt:Jt + 1])
            KT[Jt] = t

        # ---- V projection -> V position-major bf16 [128, 256] x2
        V = {}
        for pc in range(2):
            ps = psw.tile([128, 1024], F32, tag="qk")
            seed_bias(ps[:, :E], brows[("bv", l)], 128, E)
            for k in range(2):
                nc.tensor.matmul(
                    ps[:, :E],
                    XT_t[k][:, pc * 128:(pc + 1) * 128],
                    Wv_t[(l, k)][:],
                    start=False, stop=(k == 1))
            t = work.tile([128, E], BF16, tag=f"V{pc}")
            nc.vector.tensor_copy(t[:], ps[:, :E])
            V[pc] = t

        # ---- Q projection -> Qs (decay-scaled) bf16, rotations r=0..3
        Qs = {}
        for h in range(2):
            ps = psw.tile([128, 1024], F32, tag="qk")
            for k in range(2):
                nc.tensor.matmul(
                    ps[:, :qw],
                    Wq_t[(l, k)][:, h * 128:(h + 1) * 128],
                    XT_t[k][:, qlo:qhi],
                    start=(k == 0), stop=(k == 1))
            tf = work.tile([128, NPOS], F32, tag="qtmp")
            nc.vector.tensor_scalar_add(tf[:, :qw], ps[:, :qw],
                                        bq_t[l][:, h:h + 1])
            t = work.tile([128, NPOS], BF16, tag=f"Qs0{h}")
            nc.vector.tensor_mul(t[:, :qw], tf[:, :qw], D_t[h][:, qlo:qhi])
            Qs[(0, h)] = t
        for r in range(1, 4):
            for h in range(2):
                ps = psw.tile([128, 1024], F32, tag="qk")
                nc.tensor.matmul(ps[:, :qw], Prot_t[r - 1][:],
                                 Qs[(0, h)][:, :qw], start=True, stop=True)
                t = work.tile([128, NPOS], BF16, tag=f"Qs{r}{h}")
                nc.vector.tensor_copy(t[:, :qw], ps[:, :qw])
                Qs[(r, h)] = t

        # ---- attention: ST -> exp -> A@V (+ row sums), accumulated in PSUM
        # Accumulators are zeroed by DVE memset; every matmul then uses
        # start=False (pure accumulate), so scheduler order within the
        # region doesn't matter.  skip_group_check silences the group
        # bookkeeping that this pattern sidesteps.
        OT_ps = psacc.tile([128, 2, NPOS], F32, tag="ot")   # [ch128, h, q]
        RS_ps = psacc.tile([128, 2, NPOS], F32, tag="rs")
        nc.vector.memset(OT_ps[:], 0.0)
        nc.vector.memset(RS_ps[:], 0.0)
        exp_i = 0
        for J in range(2):          # key c'-quad
            for pc in range(2):     # key position chunk
                ATl = {}
                for h in range(2):
                    for r in range(4):
                        # each matmul gets its own 2KB psum zero-region
                        # (slices padded to 512 f32): region-sharing with a
                        # split start/stop group crashes the device.
                        psa = psw.tile([128, 2, 512], F32, tag="qk")
                        psb = psw.tile([128, 2, 512], F32, tag="qk")
                        for i in range(4):
                            pst = psa if i < 2 else psb
                            nc.tensor.matmul(
                                pst[:, i % 2, :qw],
                                KT[J][32 * i:32 * (i + 1),
                                      pc * 128:(pc + 1) * 128],
                                Qs[(r, h)][32 * i:32 * (i + 1), :qw],
                                start=True, stop=True,
                                tile_position=(32 * i, 0))
                        for half, pst in ((0, psa), (1, psb)):
                            at2 = atp.tile([128, 2, NPOS], BF16, tag="at")
                            if exp_i % 2 == 1:
                                ti = work.tile([128, 2, NPOS], I32, tag="ei")
                                nc.vector.tensor_scalar(ti[:, :, :qw],
                                                        pst[:, :, :qw],
                                                        SCH_A, SCH_B,
                                                        MULT, ADD)
                                nc.vector.tensor_copy(
                                    at2[:, :, :qw],
                                    ti[:, :, :qw].bitcast(F32))
                            else:
                                nc.scalar.activation(at2[:, :, :qw],
                                                     pst[:, :, :qw],
                                                     AF.Exp)
                            exp_i += 1
                            for g in range(2):
                                i = 2 * half + g
                                c = 4 * h + (i + r) % 4
                                ATl[(c, i)] = at2[:, g, :qw]
                for i in range(4):  # s-chunk (c' = 4J+i, pc)
                    cp = 4 * J + i
                    Vv = V[pc][:, 32 * cp:32 * (cp + 1)]   # [128, 32] bf16
                    for h in range(2):
                        for j in range(4):
                            nc.tensor.matmul(
                                OT_ps[32 * j:32 * (j + 1), h, :qw],
                                Vv, ATl[(4 * h + j, i)],
                                start=False, stop=False,
                                skip_group_check=True,
                                tile_position=(0, 32 * j))
                        for j in range(4):
                            nc.tensor.matmul(
                                RS_ps[32 * j:32 * j + 1, h, :qw],
                                ones_col[:], ATl[(4 * h + j, i)],
                                start=False, stop=False,
                                skip_group_check=True,
                                tile_position=(0, 32 * j))

        # ---- normalize: OT = OT * (1/RS) broadcast over the 32-row blocks
        # reciprocal once over the whole tile (unused partitions hold
        # garbage, never read), then K=1 matmuls broadcast row 32j over
        # its 32-row block.
        rcp = work.tile([128, 2, NPOS], F32, tag="rcp")
        nc.vector.reciprocal_approx_fast(rcp[:, :, :qw], RS_ps[:, :, :qw])
        RB_ps = psw.tile([128, 2, 512], F32, tag="qk")
        nc.vector.memset(RB_ps[:, :, :qw], 0.0)
        for h in range(2):
            for j in range(4):
                nc.tensor.matmul(
                    RB_ps[32 * j:32 * (j + 1), h, :qw],
                    ones32[32 * j:32 * j + 1, :32],
                    rcp[32 * j:32 * j + 1, h, :qw],
                    start=False, stop=False, skip_group_check=True,
                    tile_position=(32 * j, 32 * j))
        rb_sb = work.tile([128, 2, NPOS], F32, tag="rb_sb")
        nc.vector.tensor_copy(rb_sb[:, :, :qw], RB_ps[:, :, :qw])
        ot_t = work.tile([128, 2, NPOS], WDT, tag="OTn")
        nc.vector.tensor_tensor(ot_t[:, :, :qw], OT_ps[:, :, :qw],
                                rb_sb[:, :, :qw], MULT)
        OT_sb = {h: ot_t[:, h, :qw] for h in range(2)}

        # ---- O @ Wo + bo + residual -> LN1 -> xa
        xa = {}
        for ci, (xi, ro, nr) in enumerate(pos_chunks):
            ps = psw.tile([128, 1024], F32, tag="qk")
            seed_bias(ps[:nr, :E], brows[("bo", l)], nr, E)
            for h in range(2):
                nc.tensor.matmul(
                    ps[:nr, :E],
                    OT_sb[h][:, ci * 128:ci * 128 + nr],
                    Wo_t[(l, h)][:],
                    start=False, stop=(h == 1))
            res = work.tile([128, NPOS], F32, tag=f"res{ci}")
            nc.vector.tensor_add(res[:nr], ps[:nr, :E],
                                 X_t[xi][ro:ro + nr, :])
            t = work.tile([128, NPOS], F32, tag=f"xa{ci}")
            layernorm(res[:nr], nr, ln_b[("ln1g", l)], ln_b[("ln1b", l)],
                      t[:nr])
            xa[ci] = t

        # ---- transpose xa -> xaT channel-major
        xaT = {}
        for k in range(2):
            t = work.tile([128, NPOS], WDT, tag=f"xaT{k}")
            for ci, (_, _, nr) in enumerate(pos_chunks):
                ps = psw.tile([128, 1024], F32, tag="qk")
                nc.tensor.transpose(ps[:, :nr],
                                    xa[ci][:nr, k * 128:(k + 1) * 128],
                                    ident[:nr, :nr])
                nc.vector.tensor_copy(t[:, ci * 128:ci * 128 + nr],
                                      ps[:, :nr])
            xaT[k] = t

        # ---- FFN: H1T = relu(W1.T x + b1) channel-major bf16 [128, qw] x8
        H1T = {}
        for dk in range(8):
            ps = psw.tile([128, 1024], F32, tag="qk")
            for k in range(2):
                nc.tensor.matmul(
                    ps[:, :qw],
                    W1_t[(l, k)][:, dk * 128:(dk + 1) * 128],
                    xaT[k][:, :qw],
                    start=(k == 0), stop=(k == 1))
            t = work.tile([128, NPOS], BF16, tag=f"H1T{dk}")
            nc.vector.tensor_scalar(t[:, :qw], ps[:, :qw],
                                    b1_t[l][:, dk:dk + 1], 0.0, ADD, MAX)
            H1T[dk] = t

        # ---- FF = relu(H1 @ W2 + b2); X_next = LN2(xa + FF)
        newX = {}
        for ci, (_, _, nr) in enumerate(pos_chunks):
            ps = psw.tile([128, 1024], F32, tag="qk")
            seed_bias(ps[:nr, :E], brows[("b2", l)], nr, E)
            for dk in range(8):
                nc.tensor.matmul(
                    ps[:nr, :E],
                    H1T[dk][:, ci * 128:ci * 128 + nr],
                    W2_t[(l, dk)][:],
                    start=False, stop=(dk == 7))
            t = work.tile([128, NPOS], F32, tag=f"ff{ci}")
            nc.vector.tensor_scalar_max(t[:nr], ps[:nr, :E], 0.0)
            res2 = work.tile([128, NPOS], F32, tag=f"res2{ci}")
            nc.vector.tensor_add(res2[:nr], t[:nr], xa[ci][:nr])
            xn = const.tile([128, NPOS], F32, tag=f"Xn{l}{ci}")
            layernorm(res2[:nr], nr, ln_b[("ln2g", l)], ln_b[("ln2b", l)],
                      xn[:nr])
            newX[ci] = xn

        if l == 0:
            X_t = {0: newX[0], 1: newX[1]}
            XT_t = {}
            for k in range(2):
                t = const.tile([128, NPOS], WDT, tag=f"X1T{k}")
                for ci in range(2):
                    ps = psw.tile([128, 1024], F32, tag="qk")
                    nc.tensor.transpose(ps[:, :128],
                                        newX[ci][:, k * 128:(k + 1) * 128],
                                        ident[:])
                    nc.vector.tensor_copy(t[:, ci * 128:(ci + 1) * 128],
                                          ps[:, :128])
                XT_t[k] = t
        else:
            X2 = newX[0]  # [128, 256]: positions 128..255

    # ---------------- final LN + projection (row output) ----------------
    xf = work.tile([128, NPOS], F32, tag="xf")
    layernorm(X2[:128], 128, ln_b["lnfg"], ln_b["lnfb"], xf[:128])
    xfT = {}
    for k in range(2):
        ps = psw.tile([128, 1024], F32, tag="qk")
        nc.tensor.transpose(ps[:, :128], xf[:, k * 128:(k + 1) * 128],
                            ident[:])
        t = work.tile([128, 128], F32, tag=f"xfT{k}")
        nc.vector.tensor_copy(t[:], ps[:, :128])
        xfT[k] = t
    ps = psw.tile([128, 1024], F32, tag="qk")
    nc.tensor.matmul(ps[0:1, :128], bprow[0:1, 0:1],
                     ones_row[0:1, :128], start=True, stop=False)
    for k in range(2):
        nc.tensor.matmul(ps[0:1, :128], Wp_sb[:, k:k + 1],
                         xfT[k][:],
                         start=False, stop=(k == 1))
    ot = work.tile([1, PRED], F32, tag="outsb")
    # output = last 96 of the 128 computed positions
    nc.vector.tensor_copy(ot[:], ps[0:1, 128 - PRED:128])
    nc.sync.dma_start(out=outs["out"][:], in_=ot[:])


# ======================= host side =======================

def _rot_matrices():
    """P_r[k, m] = 1 iff k = 32*((m//32 + r) % 4) + m % 32, r = 1..3."""
    import ml_dtypes
    mats = np.zeros((3, 128, 128), np.float32)
    for r in range(1, 4):
        for m in range(128):
            mats[r - 1, 32 * ((m // 32 + r) % 4) + m % 32, m] = 1.0
    return mats.astype(ml_dtypes.bfloat16)


def _make_in_maps(inputs):
    import ml_dtypes
    f = np.float32
    bh = ml_dtypes.bfloat16
    x_enc = np.asarray(inputs["x_enc"], f)
    td = np.asarray(inputs["time_diffs"], f)

    def g(nm):
        return np.asarray(inputs[nm], f)

    # ---- PF1: pre-broadcast LN tiles
    pf1 = np.zeros((128, CF1), f)
    ln_list = [g("ln1_g")[0], g("ln1_b")[0], g("ln2_g")[0], g("ln2_b")[0],
               g("ln1_g")[1], g("ln1_b")[1], g("ln2_g")[1], g("ln2_b")[1],
               g("lnf_g"), g("lnf_b")]
    for i, v in enumerate(ln_list):
        pf1[:, 256 * i:256 * (i + 1)] = v[None, :]

    # ---- PB0: Wq/Wk/Wv + Prot + bias rows
    pb0 = np.zeros((128, CB0), bh)
    for l in range(LYR):
        for k in range(2):
            i = 2 * l + k
            pb0[:, WQ_O + 256 * i:WQ_O + 256 * (i + 1)] = \
                g("Wq")[l, 128 * k:128 * (k + 1), :].astype(bh)
            pb0[:, WK_O + 256 * i:WK_O + 256 * (i + 1)] = \
                g("Wk")[l, 128 * k:128 * (k + 1), :].astype(bh)
            pb0[:, WV_O + 256 * i:WV_O + 256 * (i + 1)] = \
                g("Wv")[l, 128 * k:128 * (k + 1), :].astype(bh)
    prot = _rot_matrices()
    for r in range(3):
        pb0[:, PROT_O + 128 * r:PROT_O + 128 * (r + 1)] = prot[r]
    for i, (nm, l) in enumerate([("bv", 0), ("bv", 1), ("bo", 0),
                                 ("bo", 1), ("b2", 0), ("b2", 1)]):
        pb0[0, BROW_O + 256 * i:BROW_O + 256 * (i + 1)] = \
            g(nm)[l].astype(bh)

    # ---- PB1: Wo + W1 + W2
    pb1 = np.zeros((128, CB1), bh)
    for l in range(LYR):
        for h in range(2):
            i = 2 * l + h
            pb1[:, WO_O + 256 * i:WO_O + 256 * (i + 1)] = \
                g("Wo")[l, 128 * h:128 * (h + 1), :].astype(bh)
        for k in range(2):
            i = 2 * l + k
            pb1[:, W1_O + 1024 * i:W1_O + 1024 * (i + 1)] = \
                g("W1")[l, 128 * k:128 * (k + 1), :].astype(bh)
        for dk in range(8):
            i = 8 * l + dk
            pb1[:, W2_O + 256 * i:W2_O + 256 * (i + 1)] = \
                g("W2")[l, 128 * dk:128 * (dk + 1), :].astype(bh)

    # ---- PF0 (per-core: contains xT and td)
    pf0_base = np.zeros((128, CF0), f)
    pf0_base[0:7, WEMB_O:WEMB_O + 256] = g("W_emb")  # [7, 256]
    pf0_base[:, BEMBPP_O:BEMBPP_O + 2] = g("b_emb").reshape(2, 128).T
    pf0_base[0, BEMBR_O:BEMBR_O + 256] = g("b_emb")
    for l in range(LYR):
        pf0_base[:, BQ_O + 2 * l:BQ_O + 2 * l + 2] = \
            g("bq")[l].reshape(2, 128).T
        pf0_base[:, BK_O + 2 * l:BK_O + 2 * l + 2] = \
            g("bk")[l].reshape(2, 128).T
        pf0_base[:, B1_O + 8 * l:B1_O + 8 * l + 8] = \
            g("b1")[l].reshape(8, 128).T
    pf0_base[:, WP_O:WP_O + 2] = g("W_proj")[:, 0].reshape(2, 128).T
    pf0_base[0, BPROJ_O] = g("b_proj")[0]

    pb0 = np.ascontiguousarray(pb0)
    pb1 = np.ascontiguousarray(pb1)
    pf1 = np.ascontiguousarray(pf1)
    maps = []
    for b in range(B):
        pf0 = pf0_base.copy()
        # td8row[0, c*256 + q] = td[b, q*8 + c]  (head-view c-major order)
        pf0[0, TD8R:TD8R + L] = td[b].reshape(256, 8).T.reshape(-1)
        pf0[0:7, XT_O:XT_O + 256] = x_enc[b, P0:P0 + NPOS, :].T
        maps.append({"PF0": np.ascontiguousarray(pf0), "PB0": pb0,
                     "PF1": pf1, "PB1": pb1})
    return maps


OUT_SHAPE = (1, PRED)


def _assemble(results, expected_like=None):
    return np.stack(
        [list(results[b].values())[0].reshape(PRED) for b in range(B)]
    ).astype(np.float32)


def _run(in_maps, check_with_sim=False, check_with_hw=True, **kw):
    from concourse.bass_test_utils import run_kernel

    n = len(in_maps)
    out_like = {"out": np.zeros(OUT_SHAPE, np.float32)}
    res = run_kernel(
        lambda tc, outs, ins: chaos_kernel(tc, outs, ins),
        None,
        in_maps if n > 1 else in_maps[0],
        output_like=[out_like] * n if n > 1 else out_like,
        bass_type=tile.TileContext,
        num_cores=n,
        check_with_sim=check_with_sim,
        check_with_hw=check_with_hw,
        trace_sim=False,
        **kw,
    )
    return res


def kernel(**inputs):
    in_maps = _make_in_maps(inputs)
    res = _run(in_maps)
    return _assemble(res.results)


# revision 35
# speedup vs baseline: 1.0483x; 1.0483x over previous
"""Trainium2 Bass kernel for nn_ChaosTransformer_22333829939822.

Key mathematical reduction (verified against the reference):
the torch-style ``view(B, H, L, E//H)`` on a [B, L, E] tensor is a raw
row-major reshape, which makes head h attend only within the 256-position
block [h*256, (h+1)*256).  The output ``dec[:, -96:, 0]`` therefore depends
only on the last 256 positions of each batch.  Each core runs one batch's
[256, 256] residual-stream transformer; attention operates on the
[2048, 32] head-view of the 256x256 block.

Sharding: data-parallel over batch B across 4 of the 8 cores (one batch
per core, fully independent, no collectives).

Layouts on device:
- residual stream X kept position-major ([pos, ch], for LayerNorm) and
  channel-major XT ([ch, pos], matmul operand).
- attention in bf16.  Scores are built KEY-major: ST_c[s, q] so that the
  exp'd tiles feed A@V directly as the moving operand with keys on the
  contraction (partition) axis.  Row sums for the softmax denominator come
  from ones-column matmuls; normalization happens once at the end (scores
  are provably tiny here: |SCALE*decay*S| < ~3, so exp needs no max shift).
- the query c-block axis lives on SBUF partitions; 4x row-packed K=32
  matmuls need the Q tile at 4 rotations of its 32-partition blocks,
  produced by permutation matmuls (host supplies the 0/1 matrices).

Perf notes (vs the first working version):
- ALL constants arrive in 4 contiguous [128, C] packs (one DMA each,
  multi-KB descriptors) instead of ~98 small dma_starts; LN gain/bias
  rows are pre-broadcast host-side.
- the decay tile is built by K=1 broadcast MATMULS from a c-major
  time-diff row (the old strided partition_broadcast expanded to ~65k
  4-byte DMA descriptors and stalled the kernel ~45us).
- softmax row-sum normalization: one reciprocal_approx_fast over the
  whole [128,2,qw] tile + broadcast matmuls (the old path did 16
  single-partition reciprocals plus a DRAM round-trip per layer).
- the output leaves as a [1, 96] row (1 descriptor) instead of [96, 1].
- every 3rd softmax-exp tile is computed on the DVE with a Schraudolph
  bitcast exp (the ACT engine is otherwise the attention-phase
  bottleneck alongside the PE; measured engines all ~80-95% busy there).
"""

import sys
import numpy as np

sys.path.insert(0, "/opt/trn_rl_repo")

import concourse.bass as bass
import concourse.tile as tile
from concourse import mybir
from concourse.masks import make_identity

F32 = mybir.dt.float32
BF16 = mybir.dt.bfloat16
I32 = mybir.dt.int32
# Schraudolph fast-exp constants: exp(x) ~= bitcast_f32(int(SCH_A*x + SCH_B)),
# max rel err ~3% on |x|<3 — same scale as fp8 A-quantization, which costs
# only ~2e-4 on the final output (numpy-simulated).  Used to offload a third
# of the softmax exps from the saturated ACT engine onto the DVE.
SCH_A = float(2 ** 23 / np.log(2))
SCH_B = float(127 * 2 ** 23 - 366393)
ADD = mybir.AluOpType.add
SUB = mybir.AluOpType.subtract
MULT = mybir.AluOpType.mult
MAX = mybir.AluOpType.max
ASR = mybir.AluOpType.arith_shift_right
AF = mybir.ActivationFunctionType

B, L, D, E, DFF, LYR, PRED = 4, 2048, 7, 256, 1024, 2, 96
FACTOR = 5.0
SCALE = 1.0 / float(np.sqrt(FACTOR))
LN_SCALE = float(np.log(SCALE))
EPS = 1e-5
P0 = L - 256          # 1792: start of the last 256-position block
QLO2 = 128            # layer-2 computes query positions [128, 256)
NPOS = 256

# ---- packed-constant column layouts (host and device share these) ----
# PF0 (fp32, urgent): td (c-major row) / xT / Wemb / small biases / proj
TD8R = 0                      # [1, 2048] td8row[0, c*256+q] = td[8q+c]
XT_O = 2048                   # [7, 256]
WEMB_O = XT_O + 256           # [7, 256]
BEMBPP_O = WEMB_O + 256       # [128, 2]
BEMBR_O = BEMBPP_O + 2        # [1, 256]
BQ_O = BEMBR_O + 256          # [128, 2] per layer
BK_O = BQ_O + 2 * LYR         # [128, 2] per layer
B1_O = BK_O + 2 * LYR         # [128, 8] per layer
WP_O = B1_O + 8 * LYR         # [128, 2]
BPROJ_O = WP_O + 2            # [1, 1]
CF0 = BPROJ_O + 1

# PF1 (fp32): 10 pre-broadcast LN tiles [128, 256] each
# order: ln1g0 ln1b0 ln2g0 ln2b0 ln1g1 ln1b1 ln2g1 ln2b1 lnfg lnfb
CF1 = 10 * 256

# PB0 (bf16, urgent): Wq/Wk/Wv [(l,k)->128x256], Prot, bias rows
WQ_O = 0                      # 4 x 256
WK_O = WQ_O + 4 * 256
WV_O = WK_O + 4 * 256
PROT_O = WV_O + 4 * 256       # 3 x 128
BROW_O = PROT_O + 3 * 128     # 6 x [1, 256] (row 0): bv0 bv1 bo0 bo1 b20 b21
CB0 = BROW_O + 6 * 256

# PB1 (bf16): Wo [(l,h)], W1 [(l,k)->128x1024], W2 [(l,dk)->128x256]
WO_O = 0                      # 4 x 256
W1_O = WO_O + 4 * 256         # 4 x 1024
W2_O = W1_O + 4 * 1024        # 16 x 256
CB1 = W2_O + 16 * 256


def chaos_kernel(tc, outs, ins):
    import contextlib

    nc = tc.nc
    with contextlib.ExitStack() as ctx:
        _chaos_body(tc, nc, ctx, outs, ins)


def _chaos_body(tc, nc, ctx, outs, ins):
    WDT = BF16
    const = ctx.enter_context(tc.tile_pool(name="const", bufs=1))
    work = ctx.enter_context(tc.tile_pool(name="work", bufs=3))
    atp = ctx.enter_context(tc.tile_pool(name="atp", bufs=24))
    psw = ctx.enter_context(tc.tile_pool(name="psw", bufs=3, space="PSUM"))
    psacc = ctx.enter_context(tc.tile_pool(name="psacc", bufs=1, space="PSUM"))

    dma = nc.sync.dma_start

    def seed_bias(ps_ap, brow_ap, m, n):
        """PSUM <- bias row broadcast over m partitions (K=1 matmul)."""
        ones = ones_row if brow_ap.dtype == F32 else ones_row_w
        nc.tensor.matmul(
            ps_ap, ones[0:1, :m], brow_ap,
            start=True, stop=False,
        )

    def layernorm(x_ap, rows, g_b, b_b, out_ap):
        st = work.tile([128, 6], F32, tag="bn_st")
        nc.vector.bn_stats(st[:rows], x_ap)
        mv = work.tile([128, 2], F32, tag="bn_mv")
        nc.vector.bn_aggr(mv[:rows], st[:rows])
        sd = work.tile([128, 1], F32, tag="bn_sd")
        nc.scalar.activation(sd[:rows], mv[:rows, 1:2], AF.Sqrt,
                             bias=eps_t[:rows])
        nc.vector.reciprocal(sd[:rows], sd[:rows])
        t = work.tile([128, NPOS], F32, tag="ln_t")
        nc.vector.tensor_scalar(t[:rows], x_ap, mv[:rows, 0:1], sd[:rows],
                                SUB, MULT)
        nc.vector.tensor_mul(t[:rows], t[:rows], g_b[:rows])
        nc.vector.tensor_add(out_ap, t[:rows], b_b[:rows])

    # ---------------- packed constant loads (urgency order) ----------------
    PF0t = const.tile([128, CF0], F32, tag="PF0")
    dma(out=PF0t[:], in_=ins["PF0"][:])
    PB0t = const.tile([128, CB0], WDT, tag="PB0")
    dma(out=PB0t[:], in_=ins["PB0"][:])
    PF1t = const.tile([128, CF1], F32, tag="PF1")
    dma(out=PF1t[:], in_=ins["PF1"][:])
    PB1t = const.tile([128, CB1], WDT, tag="PB1")
    dma(out=PB1t[:], in_=ins["PB1"][:])

    xT_sb = PF0t[0:7, XT_O:XT_O + 256]
    Wemb_sb = PF0t[0:7, WEMB_O:WEMB_O + 256]
    bemb_pp = PF0t[:, BEMBPP_O:BEMBPP_O + 2]
    bemb_r = PF0t[0:1, BEMBR_O:BEMBR_O + 256]
    bq_t = {l: PF0t[:, BQ_O + 2 * l:BQ_O + 2 * l + 2] for l in range(LYR)}
    bk_t = {l: PF0t[:, BK_O + 2 * l:BK_O + 2 * l + 2] for l in range(LYR)}
    b1_t = {l: PF0t[:, B1_O + 8 * l:B1_O + 8 * l + 8] for l in range(LYR)}
    Wp_sb = PF0t[:, WP_O:WP_O + 2]
    bprow = PF0t[0:1, BPROJ_O:BPROJ_O + 1]

    ln_names = ["ln1g0", "ln1b0", "ln2g0", "ln2b0",
                "ln1g1", "ln1b1", "ln2g1", "ln2b1", "lnfg", "lnfb"]
    ln_b = {}
    for i, nm in enumerate(ln_names):
        v = PF1t[:, 256 * i:256 * (i + 1)]
        if nm in ("lnfg", "lnfb"):
            ln_b[nm] = v
        else:
            ln_b[(nm[:4], int(nm[4]))] = v

    Wq_t, Wk_t, Wv_t, Wo_t, W1_t, W2_t = {}, {}, {}, {}, {}, {}
    for l in range(LYR):
        for k in range(2):
            i = 2 * l + k
            Wq_t[(l, k)] = PB0t[:, WQ_O + 256 * i:WQ_O + 256 * (i + 1)]
            Wk_t[(l, k)] = PB0t[:, WK_O + 256 * i:WK_O + 256 * (i + 1)]
            Wv_t[(l, k)] = PB0t[:, WV_O + 256 * i:WV_O + 256 * (i + 1)]
            W1_t[(l, k)] = PB1t[:, W1_O + 1024 * i:W1_O + 1024 * (i + 1)]
        for h in range(2):
            i = 2 * l + h
            Wo_t[(l, h)] = PB1t[:, WO_O + 256 * i:WO_O + 256 * (i + 1)]
        for dk in range(8):
            i = 8 * l + dk
            W2_t[(l, dk)] = PB1t[:, W2_O + 256 * i:W2_O + 256 * (i + 1)]
    Prot_t = {r: PB0t[:, PROT_O + 128 * r:PROT_O + 128 * (r + 1)]
              for r in range(3)}
    brows = {}
    for i, (nm, l) in enumerate([("bv", 0), ("bv", 1), ("bo", 0),
                                 ("bo", 1), ("b2", 0), ("b2", 1)]):
        brows[(nm, l)] = PB0t[0:1, BROW_O + 256 * i:BROW_O + 256 * (i + 1)]

    ident = const.tile([128, 128], F32, tag="ident")
    make_identity(nc, ident[:])
    ones_col = const.tile([128, 1], BF16, tag="ones_col")
    nc.vector.memset(ones_col[:], 1.0)
    ones_row = const.tile([1, 128], F32, tag="ones_row")
    nc.vector.memset(ones_row[:], 1.0)
    ones_row_w = const.tile([1, 128], WDT, tag="ones_row_w")
    nc.vector.memset(ones_row_w[:], 1.0)
    eps_t = const.tile([128, 1], F32, tag="eps")
    nc.vector.memset(eps_t[:], EPS)
    ones32 = const.tile([128, 32], F32, tag="ones32")
    nc.vector.memset(ones32[:], 1.0)
    lnsc_t = const.tile([128, 1], F32, tag="lnsc")
    nc.vector.memset(lnsc_t[:], LN_SCALE)

    # ---------------- decay tiles via K=1 broadcast matmuls ----------------
    # D_t[h][32j+d, q] = SCALE * exp(-td[q*8 + (4h+j)]/F)
    #                  = exp(-td8row[(4h+j)*256 + q]/F + ln SCALE)
    T_ps = psw.tile([128, 2, 512], F32, tag="qk")
    nc.vector.memset(T_ps[:, :, :NPOS], 0.0)
    for h in range(2):
        for j in range(4):
            c = 4 * h + j
            nc.tensor.matmul(
                T_ps[32 * j:32 * (j + 1), h, :NPOS],
                ones_row[0:1, :32],
                PF0t[0:1, TD8R + c * NPOS:TD8R + (c + 1) * NPOS],
                start=False, stop=False, skip_group_check=True,
                tile_position=(0, 32 * j))
    D_t = {}
    for h in range(2):
        t = const.tile([128, NPOS], F32, tag=f"D{h}")
        nc.scalar.activation(t[:], T_ps[:, h, :NPOS], AF.Exp,
                             bias=lnsc_t[:], scale=-1.0 / FACTOR)
        D_t[h] = t

    # ---------------- embedding ----------------
    X_t, XT_t = {}, {}
    for p in range(2):  # position-major X
        ps = psw.tile([128, 1024], F32, tag="qk")
        seed_bias(ps[:, :E], bemb_r, 128, E)
        nc.tensor.matmul(ps[:, :E],
                         xT_sb[:, p * 128:(p + 1) * 128],
                         Wemb_sb[:], start=False, stop=True)
        t = const.tile([128, NPOS], F32, tag=f"X{p}")
        nc.vector.tensor_copy(t[:], ps[:, :E])
        X_t[p] = t
    for k in range(2):  # channel-major XT
        ps = psw.tile([128, 1024], F32, tag="qk")
        nc.tensor.matmul(ps[:, :NPOS],
                         Wemb_sb[:, k * 128:(k + 1) * 128],
                         xT_sb[:], start=True, stop=True)
        t = const.tile([128, NPOS], WDT, tag=f"XT{k}")
        nc.vector.tensor_scalar_add(t[:], ps[:, :NPOS], bemb_pp[:, k:k + 1])
        XT_t[k] = t

    # ---------------- transformer layers ----------------
    for l in range(LYR):
        qlo, qhi = (0, NPOS) if l == 0 else (QLO2, NPOS)
        qw = qhi - qlo
        pos_chunks = ([(0, 0, 128), (1, 0, 128)] if l == 0
                      else [(1, 0, 128)])
        # (X-tile index, row offset within tile, nrows) for output positions

        # ---- K projection -> KT channel-major bf16 [128, 256] x2
        KT = {}
        for Jt in range(2):
            ps = psw.tile([128, 1024], F32, tag="qk")
            for k in range(2):
                nc.tensor.matmul(
                    ps[:, :NPOS],
                    Wk_t[(l, k)][:, Jt * 128:(Jt + 1) * 128],
                    XT_t[k][:],
                    start=(k == 0), stop=(k == 1))
            t = work.tile([128, NPOS], BF16, tag=f"KT{Jt}")
            nc.vector.tensor_scalar_add(t[:], ps[:, :NPOS], bk_t[l][:, Jt:Jt + 1])
            KT[Jt] = t

        # ---- V projection -> V position-major bf16 [128, 256] x2
        V = {}
        for pc in range(2):
            ps = psw.tile([128, 1024], F32, tag="qk")
            seed_bias(ps[:, :E], brows[("bv", l)], 128, E)
            for k in range(2):
                nc.tensor.matmul(
                    ps[:, :E],
                    XT_t[k][:, pc * 128:(pc + 1) * 128],
                    Wv_t[(l, k)][:],
                    start=False, stop=(k == 1))
            t = work.tile([128, E], BF16, tag=f"V{pc}")
            nc.vector.tensor_copy(t[:], ps[:, :E])
            V[pc] = t

        # ---- Q projection -> Qs (decay-scaled) bf16, rotations r=0..3
        Qs = {}
        for h in range(2):
            ps = psw.tile([128, 1024], F32, tag="qk")
            for k in range(2):
                nc.tensor.matmul(
                    ps[:, :qw],
                    Wq_t[(l, k)][:, h * 128:(h + 1) * 128],
                    XT_t[k][:, qlo:qhi],
                    start=(k == 0), stop=(k == 1))
            tf = work.tile([128, NPOS], F32, tag="qtmp")
            nc.vector.tensor_scalar_add(tf[:, :qw], ps[:, :qw],
                                        bq_t[l][:, h:h + 1])
            t = work.tile([128, NPOS], BF16, tag=f"Qs0{h}")
            nc.vector.tensor_mul(t[:, :qw], tf[:, :qw], D_t[h][:, qlo:qhi])
            Qs[(0, h)] = t
        for r in range(1, 4):
            for h in range(2):
                ps = psw.tile([128, 1024], F32, tag="qk")
                nc.tensor.matmul(ps[:, :qw], Prot_t[r - 1][:],
                                 Qs[(0, h)][:, :qw], start=True, stop=True)
                t = work.tile([128, NPOS], BF16, tag=f"Qs{r}{h}")
                nc.vector.tensor_copy(t[:, :qw], ps[:, :qw])
                Qs[(r, h)] = t

        # ---- attention: ST -> exp -> A@V (+ row sums), accumulated in PSUM
        # Accumulators are zeroed by DVE memset; every matmul then uses
        # start=False (pure accumulate), so scheduler order within the
        # region doesn't matter.  skip_group_check silences the group
        # bookkeeping that this pattern sidesteps.
        OT_ps = psacc.tile([128, 2, NPOS], F32, tag="ot")   # [ch128, h, q]
        RS_ps = psacc.tile([128, 2, NPOS], F32, tag="rs")
        nc.vector.memset(OT_ps[:], 0.0)
        nc.vector.memset(RS_ps[:], 0.0)
        exp_i = 0
        for J in range(2):          # key c'-quad
            for pc in range(2):     # key position chunk
                ATl = {}
                for h in range(2):
                    for r in range(4):
                        # each matmul gets its own 2KB psum zero-region
                        # (slices padded to 512 f32): region-sharing with a
                        # split start/stop group crashes the device.  One
                        # 2-matmul tile per exp (not a psa/psb pair) so the
                        # 3-buffer psw ring pipelines deeper and the PE is
                        # less tightly coupled to the exp engines.
                        for half in range(2):
                            pst = psw.tile([128, 2, 512], F32, tag="qk")
                            for g in range(2):
                                i = 2 * half + g
                                nc.tensor.matmul(
                                    pst[:, g, :qw],
                                    KT[J][32 * i:32 * (i + 1),
                                          pc * 128:(pc + 1) * 128],
                                    Qs[(r, h)][32 * i:32 * (i + 1), :qw],
                                    start=True, stop=True,
                                    tile_position=(32 * i, 0))
                            at2 = atp.tile([128, 2, NPOS], BF16, tag="at")
                            if exp_i % 3 == 2:
                                ti = work.tile([128, 2, NPOS], I32, tag="ei")
                                nc.vector.tensor_scalar(ti[:, :, :qw],
                                                        pst[:, :, :qw],
                                                        SCH_A, SCH_B,
                                                        MULT, ADD)
                                nc.vector.tensor_copy(
                                    at2[:, :, :qw],
                                    ti[:, :, :qw].bitcast(F32))
                            else:
                                nc.scalar.activation(at2[:, :, :qw],
                                                     pst[:, :, :qw],
                                                     AF.Exp)
                            exp_i += 1
                            for g in range(2):
                                i = 2 * half + g
                                c = 4 * h + (i + r) % 4
                                ATl[(c, i)] = at2[:, g, :qw]
                for i in range(4):  # s-chunk (c' = 4J+i, pc)
                    cp = 4 * J + i
                    Vv = V[pc][:, 32 * cp:32 * (cp + 1)]   # [128, 32] bf16
                    for h in range(2):
                        for j in range(4):
                            nc.tensor.matmul(
                                OT_ps[32 * j:32 * (j + 1), h, :qw],
                                Vv, ATl[(4 * h + j, i)],
                                start=False, stop=False,
                                skip_group_check=True,
                                tile_position=(0, 32 * j))
                        for j in range(4):
                            nc.tensor.matmul(
                                RS_ps[32 * j:32 * j + 1, h, :qw],
                                ones_col[:], ATl[(4 * h + j, i)],
                                start=False, stop=False,
                                skip_group_check=True,
                                tile_position=(0, 32 * j))

        # ---- normalize: OT = OT * (1/RS) broadcast over the 32-row blocks
        # reciprocal once over the whole tile (unused partitions hold
        # garbage, never read), then K=1 matmuls broadcast row 32j over
        # its 32-row block.
        rcp = work.tile([128, 2, NPOS], F32, tag="rcp")
        nc.vector.reciprocal_approx_fast(rcp[:, :, :qw], RS_ps[:, :, :qw])
        RB_ps = psw.tile([128, 2, 512], F32, tag="qk")
        nc.vector.memset(RB_ps[:, :, :qw], 0.0)
        for h in range(2):
            for j in range(4):
                nc.tensor.matmul(
                    RB_ps[32 * j:32 * (j + 1), h, :qw],
                    ones32[32 * j:32 * j + 1, :32],
                    rcp[32 * j:32 * j + 1, h, :qw],
                    start=False, stop=False, skip_group_check=True,
                    tile_position=(32 * j, 32 * j))
        rb_sb = work.tile([128, 2, NPOS], F32, tag="rb_sb")
        nc.vector.tensor_copy(rb_sb[:, :, :qw], RB_ps[:, :, :qw])
        ot_t = work.tile([128, 2, NPOS], WDT, tag="OTn")
        nc.vector.tensor_tensor(ot_t[:, :, :qw], OT_ps[:, :, :qw],
                                rb_sb[:, :, :qw], MULT)
        OT_sb = {h: ot_t[:, h, :qw] for h in range(2)}

        # ---- O @ Wo + bo + residual -> LN1 -> xa
        xa = {}
        for ci, (xi, ro, nr) in enumerate(pos_chunks):
            ps = psw.tile([128, 1024], F32, tag="qk")
            seed_bias(ps[:nr, :E], brows[("bo", l)], nr, E)
            for h in range(2):
                nc.tensor.matmul(
                    ps[:nr, :E],
                    OT_sb[h][:, ci * 128:ci * 128 + nr],
                    Wo_t[(l, h)][:],
                    start=False, stop=(h == 1))
            res = work.tile([128, NPOS], F32, tag=f"res{ci}")
            nc.vector.tensor_add(res[:nr], ps[:nr, :E],
                                 X_t[xi][ro:ro + nr, :])
            t = work.tile([128, NPOS], F32, tag=f"xa{ci}")
            layernorm(res[:nr], nr, ln_b[("ln1g", l)], ln_b[("ln1b", l)],
                      t[:nr])
            xa[ci] = t

        # ---- transpose xa -> xaT channel-major
        xaT = {}
        for k in range(2):
            t = work.tile([128, NPOS], WDT, tag=f"xaT{k}")
            for ci, (_, _, nr) in enumerate(pos_chunks):
                ps = psw.tile([128, 1024], F32, tag="qk")
                nc.tensor.transpose(ps[:, :nr],
                                    xa[ci][:nr, k * 128:(k + 1) * 128],
                                    ident[:nr, :nr])
                nc.vector.tensor_copy(t[:, ci * 128:ci * 128 + nr],
                                      ps[:, :nr])
            xaT[k] = t

        # ---- FFN: H1T = relu(W1.T x + b1) channel-major bf16 [128, qw] x8
        H1T = {}
        for dk in range(8):
            ps = psw.tile([128, 1024], F32, tag="qk")
            for k in range(2):
                nc.tensor.matmul(
                    ps[:, :qw],
                    W1_t[(l, k)][:, dk * 128:(dk + 1) * 128],
                    xaT[k][:, :qw],
                    start=(k == 0), stop=(k == 1))
            t = work.tile([128, NPOS], BF16, tag=f"H1T{dk}")
            nc.vector.tensor_scalar(t[:, :qw], ps[:, :qw],
                                    b1_t[l][:, dk:dk + 1], 0.0, ADD, MAX)
            H1T[dk] = t

        # ---- FF = relu(H1 @ W2 + b2); X_next = LN2(xa + FF)
        newX = {}
        for ci, (_, _, nr) in enumerate(pos_chunks):
            ps = psw.tile([128, 1024], F32, tag="qk")
            seed_bias(ps[:nr, :E], brows[("b2", l)], nr, E)
            for dk in range(8):
                nc.tensor.matmul(
                    ps[:nr, :E],
                    H1T[dk][:, ci * 128:ci * 128 + nr],
                    W2_t[(l, dk)][:],
                    start=False, stop=(dk == 7))
            t = work.tile([128, NPOS], F32, tag=f"ff{ci}")
            nc.vector.tensor_scalar_max(t[:nr], ps[:nr, :E], 0.0)
            res2 = work.tile([128, NPOS], F32, tag=f"res2{ci}")
            nc.vector.tensor_add(res2[:nr], t[:nr], xa[ci][:nr])
            xn = const.tile([128, NPOS], F32, tag=f"Xn{l}{ci}")
            layernorm(res2[:nr], nr, ln_b[("ln2g", l)], ln_b[("ln2b", l)],
                      xn[:nr])
            newX[ci] = xn

        if l == 0:
            X_t = {0: newX[0], 1: newX[1]}
            XT_t = {}
            for k in range(2):
                t = const.tile([128, NPOS], WDT, tag=f"X1T{k}")
                for ci in range(2):
                    ps = psw.tile([128, 1024], F32, tag="qk")
                    nc.tensor.transpose(ps[:, :128],
                                        newX[ci][:, k * 128:(k + 1) * 128],
                                        ident[:])
                    nc.vector.tensor_copy(t[:, ci * 128:(ci + 1) * 128],
                                          ps[:, :128])
                XT_t[k] = t
        else:
            X2 = newX[0]  # [128, 256]: positions 128..255

    # ---------------- final LN + projection (row output) ----------------
    xf = work.tile([128, NPOS], F32, tag="xf")
    layernorm(X2[:128], 128, ln_b["lnfg"], ln_b["lnfb"], xf[:128])
    xfT = {}
    for k in range(2):
        ps = psw.tile([128, 1024], F32, tag="qk")
        nc.tensor.transpose(ps[:, :128], xf[:, k * 128:(k + 1) * 128],
                            ident[:])
        t = work.tile([128, 128], F32, tag=f"xfT{k}")
        nc.vector.tensor_copy(t[:], ps[:, :128])
        xfT[k] = t
    ps = psw.tile([128, 1024], F32, tag="qk")
    nc.tensor.matmul(ps[0:1, :128], bprow[0:1, 0:1],
                     ones_row[0:1, :128], start=True, stop=False)
    for k in range(2):
        nc.tensor.matmul(ps[0:1, :128], Wp_sb[:, k:k + 1],
                         xfT[k][:],
                         start=False, stop=(k == 1))
    ot = work.tile([1, PRED], F32, tag="outsb")
    # output = last 96 of the 128 computed positions
    nc.vector.tensor_copy(ot[:], ps[0:1, 128 - PRED:128])
    nc.sync.dma_start(out=outs["out"][:], in_=ot[:])


# ======================= host side =======================

def _rot_matrices():
    """P_r[k, m] = 1 iff k = 32*((m//32 + r) % 4) + m % 32, r = 1..3."""
    import ml_dtypes
    mats = np.zeros((3, 128, 128), np.float32)
    for r in range(1, 4):
        for m in range(128):
            mats[r - 1, 32 * ((m // 32 + r) % 4) + m % 32, m] = 1.0
    return mats.astype(ml_dtypes.bfloat16)


def _make_in_maps(inputs):
    import ml_dtypes
    f = np.float32
    bh = ml_dtypes.bfloat16
    x_enc = np.asarray(inputs["x_enc"], f)
    td = np.asarray(inputs["time_diffs"], f)

    def g(nm):
        return np.asarray(inputs[nm], f)

    # ---- PF1: pre-broadcast LN tiles
    pf1 = np.zeros((128, CF1), f)
    ln_list = [g("ln1_g")[0], g("ln1_b")[0], g("ln2_g")[0], g("ln2_b")[0],
               g("ln1_g")[1], g("ln1_b")[1], g("ln2_g")[1], g("ln2_b")[1],
               g("lnf_g"), g("lnf_b")]
    for i, v in enumerate(ln_list):
        pf1[:, 256 * i:256 * (i + 1)] = v[None, :]

    # ---- PB0: Wq/Wk/Wv + Prot + bias rows
    pb0 = np.zeros((128, CB0), bh)
    for l in range(LYR):
        for k in range(2):
            i = 2 * l + k
            pb0[:, WQ_O + 256 * i:WQ_O + 256 * (i + 1)] = \
                g("Wq")[l, 128 * k:128 * (k + 1), :].astype(bh)
            pb0[:, WK_O + 256 * i:WK_O + 256 * (i + 1)] = \
                g("Wk")[l, 128 * k:128 * (k + 1), :].astype(bh)
            pb0[:, WV_O + 256 * i:WV_O + 256 * (i + 1)] = \
                g("Wv")[l, 128 * k:128 * (k + 1), :].astype(bh)
    prot = _rot_matrices()
    for r in range(3):
        pb0[:, PROT_O + 128 * r:PROT_O + 128 * (r + 1)] = prot[r]
    for i, (nm, l) in enumerate([("bv", 0), ("bv", 1), ("bo", 0),
                                 ("bo", 1), ("b2", 0), ("b2", 1)]):
        pb0[0, BROW_O + 256 * i:BROW_O + 256 * (i + 1)] = \
            g(nm)[l].astype(bh)

    # ---- PB1: Wo + W1 + W2
    pb1 = np.zeros((128, CB1), bh)
    for l in range(LYR):
        for h in range(2):
            i = 2 * l + h
            pb1[:, WO_O + 256 * i:WO_O + 256 * (i + 1)] = \
                g("Wo")[l, 128 * h:128 * (h + 1), :].astype(bh)
        for k in range(2):
            i = 2 * l + k
            pb1[:, W1_O + 1024 * i:W1_O + 1024 * (i + 1)] = \
                g("W1")[l, 128 * k:128 * (k + 1), :].astype(bh)
        for dk in range(8):
            i = 8 * l + dk
            pb1[:, W2_O + 256 * i:W2_O + 256 * (i + 1)] = \
                g("W2")[l, 128 * dk:128 * (dk + 1), :].astype(bh)

    # ---- PF0 (per-core: contains xT and td)
    pf0_base = np.zeros((128, CF0), f)
    pf0_base[0:7, WEMB_O:WEMB_O + 256] = g("W_emb")  # [7, 256]
    pf0_base[:, BEMBPP_O:BEMBPP_O + 2] = g("b_emb").reshape(2, 128).T
    pf0_base[0, BEMBR_O:BEMBR_O + 256] = g("b_emb")
    for l in range(LYR):
        pf0_base[:, BQ_O + 2 * l:BQ_O + 2 * l + 2] = \
            g("bq")[l].reshape(2, 128).T
        pf0_base[:, BK_O + 2 * l:BK_O + 2 * l + 2] = \
            g("bk")[l].reshape(2, 128).T
        pf0_base[:, B1_O + 8 * l:B1_O + 8 * l + 8] = \
            g("b1")[l].reshape(8, 128).T
    pf0_base[:, WP_O:WP_O + 2] = g("W_proj")[:, 0].reshape(2, 128).T
    pf0_base[0, BPROJ_O] = g("b_proj")[0]

    pb0 = np.ascontiguousarray(pb0)
    pb1 = np.ascontiguousarray(pb1)
    pf1 = np.ascontiguousarray(pf1)
    maps = []
    for b in range(B):
        pf0 = pf0_base.copy()
        # td8row[0, c*256 + q] = td[b, q*8 + c]  (head-view c-major order)
        pf0[0, TD8R:TD8R + L] = td[b].reshape(256, 8).T.reshape(-1)
        pf0[0:7, XT_O:XT_O + 256] = x_enc[b, P0:P0 + NPOS, :].T
        maps.append({"PF0": np.ascontiguousarray(pf0), "PB0": pb0,
                     "PF1": pf1, "PB1": pb1})
    return maps


OUT_SHAPE = (1, PRED)


def _assemble(results, expected_like=None):
    return np.stack(
        [list(results[b].values())[0].reshape(PRED) for b in range(B)]
    ).astype(np.float32)


def _run(in_maps, check_with_sim=False, check_with_hw=True, **kw):
    from concourse.bass_test_utils import run_kernel

    n = len(in_maps)
    out_like = {"out": np.zeros(OUT_SHAPE, np.float32)}
    res = run_kernel(
        lambda tc, outs, ins: chaos_kernel(tc, outs, ins),
        None,
        in_maps if n > 1 else in_maps[0],
        output_like=[out_like] * n if n > 1 else out_like,
        bass_type=tile.TileContext,
        num_cores=n,
        check_with_sim=check_with_sim,
        check_with_hw=check_with_hw,
        trace_sim=False,
        **kw,
    )
    return res


def kernel(**inputs):
    in_maps = _make_in_maps(inputs)
    res = _run(in_maps)
    return _assemble(res.results)


# revision 36
# speedup vs baseline: 1.0892x; 1.0390x over previous
"""Trainium2 Bass kernel for nn_ChaosTransformer_22333829939822.

Key mathematical reduction (verified against the reference):
the torch-style ``view(B, H, L, E//H)`` on a [B, L, E] tensor is a raw
row-major reshape, which makes head h attend only within the 256-position
block [h*256, (h+1)*256).  The output ``dec[:, -96:, 0]`` therefore depends
only on the last 256 positions of each batch.  Each core runs one batch's
[256, 256] residual-stream transformer; attention operates on the
[2048, 32] head-view of the 256x256 block.

Sharding: data-parallel over batch B across 4 of the 8 cores (one batch
per core, fully independent, no collectives).

Layouts on device:
- residual stream X kept position-major ([pos, ch], for LayerNorm) and
  channel-major XT ([ch, pos], matmul operand).
- attention in bf16.  Scores are built KEY-major: ST_c[s, q] so that the
  exp'd tiles feed A@V directly as the moving operand with keys on the
  contraction (partition) axis.  Row sums for the softmax denominator come
  from ones-column matmuls; normalization happens once at the end (scores
  are provably tiny here: |SCALE*decay*S| < ~3, so exp needs no max shift).
- the query c-block axis lives on SBUF partitions; 4x row-packed K=32
  matmuls need the Q tile at 4 rotations of its 32-partition blocks,
  produced by permutation matmuls (host supplies the 0/1 matrices).

Perf notes (vs the first working version):
- ALL constants arrive in 4 contiguous [128, C] packs (one DMA each,
  multi-KB descriptors) instead of ~98 small dma_starts; LN gain/bias
  rows are pre-broadcast host-side.
- the decay tile is built by K=1 broadcast MATMULS from a c-major
  time-diff row (the old strided partition_broadcast expanded to ~65k
  4-byte DMA descriptors and stalled the kernel ~45us).
- softmax row-sum normalization: one reciprocal_approx_fast over the
  whole [128,2,qw] tile + broadcast matmuls (the old path did 16
  single-partition reciprocals plus a DRAM round-trip per layer).
- the output leaves as a [1, 96] row (1 descriptor) instead of [96, 1].
- every 3rd softmax-exp tile is computed on the DVE with a Schraudolph
  bitcast exp (the ACT engine is otherwise the attention-phase
  bottleneck alongside the PE; measured engines all ~80-95% busy there).
"""

import sys
import numpy as np

sys.path.insert(0, "/opt/trn_rl_repo")

import concourse.bass as bass
import concourse.tile as tile
from concourse import mybir
from concourse.masks import make_identity

F32 = mybir.dt.float32
BF16 = mybir.dt.bfloat16
I32 = mybir.dt.int32
# Schraudolph fast-exp constants: exp(x) ~= bitcast_f32(int(SCH_A*x + SCH_B)),
# max rel err ~3% on |x|<3 — same scale as fp8 A-quantization, which costs
# only ~2e-4 on the final output (numpy-simulated).  Used to offload a third
# of the softmax exps from the saturated ACT engine onto the DVE.
SCH_A = float(2 ** 23 / np.log(2))
SCH_B = float(127 * 2 ** 23 - 366393)
ADD = mybir.AluOpType.add
SUB = mybir.AluOpType.subtract
MULT = mybir.AluOpType.mult
MAX = mybir.AluOpType.max
ASR = mybir.AluOpType.arith_shift_right
AF = mybir.ActivationFunctionType

B, L, D, E, DFF, LYR, PRED = 4, 2048, 7, 256, 1024, 2, 96
FACTOR = 5.0
SCALE = 1.0 / float(np.sqrt(FACTOR))
LN_SCALE = float(np.log(SCALE))
EPS = 1e-5
P0 = L - 256          # 1792: start of the last 256-position block
QLO2 = 160            # layer-2 computes query positions [160, 256) = the
                      # 96 the output needs; the residual rows are extracted
                      # to partition base 0 with an identity-slice matmul
NPOS = 256

# ---- packed-constant column layouts (host and device share these) ----
# PF0 (fp32, urgent): td (c-major row) / xT / Wemb / small biases / proj
TD8R = 0                      # [1, 2048] td8row[0, c*256+q] = td[8q+c]
XT_O = 2048                   # [7, 256]
WEMB_O = XT_O + 256           # [7, 256]
BEMBPP_O = WEMB_O + 256       # [128, 2]
BEMBR_O = BEMBPP_O + 2        # [1, 256]
BQ_O = BEMBR_O + 256          # [128, 2] per layer
BK_O = BQ_O + 2 * LYR         # [128, 2] per layer
B1_O = BK_O + 2 * LYR         # [128, 8] per layer
WP_O = B1_O + 8 * LYR         # [128, 2]
BPROJ_O = WP_O + 2            # [1, 1]
CF0 = BPROJ_O + 1

# PF1 (fp32): 10 pre-broadcast LN tiles [128, 256] each
# order: ln1g0 ln1b0 ln2g0 ln2b0 ln1g1 ln1b1 ln2g1 ln2b1 lnfg lnfb
CF1 = 10 * 256

# PB0 (bf16, urgent): Wq/Wk/Wv [(l,k)->128x256], Prot, bias rows
WQ_O = 0                      # 4 x 256
WK_O = WQ_O + 4 * 256
WV_O = WK_O + 4 * 256
PROT_O = WV_O + 4 * 256       # 3 x 128
BROW_O = PROT_O + 3 * 128     # 6 x [1, 256] (row 0): bv0 bv1 bo0 bo1 b20 b21
CB0 = BROW_O + 6 * 256

# PB1 (bf16): Wo [(l,h)], W1 [(l,k)->128x1024], W2 [(l,dk)->128x256]
WO_O = 0                      # 4 x 256
W1_O = WO_O + 4 * 256         # 4 x 1024
W2_O = W1_O + 4 * 1024        # 16 x 256
CB1 = W2_O + 16 * 256


def chaos_kernel(tc, outs, ins):
    import contextlib

    nc = tc.nc
    with contextlib.ExitStack() as ctx:
        _chaos_body(tc, nc, ctx, outs, ins)


def _chaos_body(tc, nc, ctx, outs, ins):
    WDT = BF16
    const = ctx.enter_context(tc.tile_pool(name="const", bufs=1))
    work = ctx.enter_context(tc.tile_pool(name="work", bufs=3))
    atp = ctx.enter_context(tc.tile_pool(name="atp", bufs=24))
    psw = ctx.enter_context(tc.tile_pool(name="psw", bufs=3, space="PSUM"))
    psacc = ctx.enter_context(tc.tile_pool(name="psacc", bufs=1, space="PSUM"))

    dma = nc.sync.dma_start

    def seed_bias(ps_ap, brow_ap, m, n):
        """PSUM <- bias row broadcast over m partitions (K=1 matmul)."""
        ones = ones_row if brow_ap.dtype == F32 else ones_row_w
        nc.tensor.matmul(
            ps_ap, ones[0:1, :m], brow_ap,
            start=True, stop=False,
        )

    def layernorm(x_ap, rows, g_b, b_b, out_ap):
        st = work.tile([128, 6], F32, tag="bn_st")
        nc.vector.bn_stats(st[:rows], x_ap)
        mv = work.tile([128, 2], F32, tag="bn_mv")
        nc.vector.bn_aggr(mv[:rows], st[:rows])
        sd = work.tile([128, 1], F32, tag="bn_sd")
        nc.scalar.activation(sd[:rows], mv[:rows, 1:2], AF.Sqrt,
                             bias=eps_t[:rows])
        nc.vector.reciprocal(sd[:rows], sd[:rows])
        t = work.tile([128, NPOS], F32, tag="ln_t")
        nc.vector.tensor_scalar(t[:rows], x_ap, mv[:rows, 0:1], sd[:rows],
                                SUB, MULT)
        nc.vector.tensor_mul(t[:rows], t[:rows], g_b[:rows])
        nc.vector.tensor_add(out_ap, t[:rows], b_b[:rows])

    # ---------------- packed constant loads (urgency order) ----------------
    PF0t = const.tile([128, CF0], F32, tag="PF0")
    dma(out=PF0t[:], in_=ins["PF0"][:])
    PB0t = const.tile([128, CB0], WDT, tag="PB0")
    dma(out=PB0t[:], in_=ins["PB0"][:])
    PF1t = const.tile([128, CF1], F32, tag="PF1")
    dma(out=PF1t[:], in_=ins["PF1"][:])
    PB1t = const.tile([128, CB1], WDT, tag="PB1")
    dma(out=PB1t[:], in_=ins["PB1"][:])

    xT_sb = PF0t[0:7, XT_O:XT_O + 256]
    Wemb_sb = PF0t[0:7, WEMB_O:WEMB_O + 256]
    bemb_pp = PF0t[:, BEMBPP_O:BEMBPP_O + 2]
    bemb_r = PF0t[0:1, BEMBR_O:BEMBR_O + 256]
    bq_t = {l: PF0t[:, BQ_O + 2 * l:BQ_O + 2 * l + 2] for l in range(LYR)}
    bk_t = {l: PF0t[:, BK_O + 2 * l:BK_O + 2 * l + 2] for l in range(LYR)}
    b1_t = {l: PF0t[:, B1_O + 8 * l:B1_O + 8 * l + 8] for l in range(LYR)}
    Wp_sb = PF0t[:, WP_O:WP_O + 2]
    bprow = PF0t[0:1, BPROJ_O:BPROJ_O + 1]

    ln_names = ["ln1g0", "ln1b0", "ln2g0", "ln2b0",
                "ln1g1", "ln1b1", "ln2g1", "ln2b1", "lnfg", "lnfb"]
    ln_b = {}
    for i, nm in enumerate(ln_names):
        v = PF1t[:, 256 * i:256 * (i + 1)]
        if nm in ("lnfg", "lnfb"):
            ln_b[nm] = v
        else:
            ln_b[(nm[:4], int(nm[4]))] = v

    Wq_t, Wk_t, Wv_t, Wo_t, W1_t, W2_t = {}, {}, {}, {}, {}, {}
    for l in range(LYR):
        for k in range(2):
            i = 2 * l + k
            Wq_t[(l, k)] = PB0t[:, WQ_O + 256 * i:WQ_O + 256 * (i + 1)]
            Wk_t[(l, k)] = PB0t[:, WK_O + 256 * i:WK_O + 256 * (i + 1)]
            Wv_t[(l, k)] = PB0t[:, WV_O + 256 * i:WV_O + 256 * (i + 1)]
            W1_t[(l, k)] = PB1t[:, W1_O + 1024 * i:W1_O + 1024 * (i + 1)]
        for h in range(2):
            i = 2 * l + h
            Wo_t[(l, h)] = PB1t[:, WO_O + 256 * i:WO_O + 256 * (i + 1)]
        for dk in range(8):
            i = 8 * l + dk
            W2_t[(l, dk)] = PB1t[:, W2_O + 256 * i:W2_O + 256 * (i + 1)]
    Prot_t = {r: PB0t[:, PROT_O + 128 * r:PROT_O + 128 * (r + 1)]
              for r in range(3)}
    brows = {}
    for i, (nm, l) in enumerate([("bv", 0), ("bv", 1), ("bo", 0),
                                 ("bo", 1), ("b2", 0), ("b2", 1)]):
        brows[(nm, l)] = PB0t[0:1, BROW_O + 256 * i:BROW_O + 256 * (i + 1)]

    ident = const.tile([128, 128], F32, tag="ident")
    make_identity(nc, ident[:])
    ones_col = const.tile([128, 1], BF16, tag="ones_col")
    nc.vector.memset(ones_col[:], 1.0)
    ones_row = const.tile([1, 128], F32, tag="ones_row")
    nc.vector.memset(ones_row[:], 1.0)
    ones_row_w = const.tile([1, 128], WDT, tag="ones_row_w")
    nc.vector.memset(ones_row_w[:], 1.0)
    eps_t = const.tile([128, 1], F32, tag="eps")
    nc.vector.memset(eps_t[:], EPS)
    ones32 = const.tile([128, 32], F32, tag="ones32")
    nc.vector.memset(ones32[:], 1.0)
    lnsc_t = const.tile([128, 1], F32, tag="lnsc")
    nc.vector.memset(lnsc_t[:], LN_SCALE)

    # ---------------- decay tiles via K=1 broadcast matmuls ----------------
    # D_t[h][32j+d, q] = SCALE * exp(-td[q*8 + (4h+j)]/F)
    #                  = exp(-td8row[(4h+j)*256 + q]/F + ln SCALE)
    T_ps = psw.tile([128, 2, 512], F32, tag="qk")
    nc.vector.memset(T_ps[:, :, :NPOS], 0.0)
    for h in range(2):
        for j in range(4):
            c = 4 * h + j
            nc.tensor.matmul(
                T_ps[32 * j:32 * (j + 1), h, :NPOS],
                ones_row[0:1, :32],
                PF0t[0:1, TD8R + c * NPOS:TD8R + (c + 1) * NPOS],
                start=False, stop=False, skip_group_check=True,
                tile_position=(0, 32 * j))
    D_t = {}
    for h in range(2):
        t = const.tile([128, NPOS], F32, tag=f"D{h}")
        nc.scalar.activation(t[:], T_ps[:, h, :NPOS], AF.Exp,
                             bias=lnsc_t[:], scale=-1.0 / FACTOR)
        D_t[h] = t

    # ---------------- embedding ----------------
    X_t, XT_t = {}, {}
    for p in range(2):  # position-major X
        ps = psw.tile([128, 1024], F32, tag="qk")
        seed_bias(ps[:, :E], bemb_r, 128, E)
        nc.tensor.matmul(ps[:, :E],
                         xT_sb[:, p * 128:(p + 1) * 128],
                         Wemb_sb[:], start=False, stop=True)
        t = const.tile([128, NPOS], F32, tag=f"X{p}")
        nc.vector.tensor_copy(t[:], ps[:, :E])
        X_t[p] = t
    for k in range(2):  # channel-major XT
        ps = psw.tile([128, 1024], F32, tag="qk")
        nc.tensor.matmul(ps[:, :NPOS],
                         Wemb_sb[:, k * 128:(k + 1) * 128],
                         xT_sb[:], start=True, stop=True)
        t = const.tile([128, NPOS], WDT, tag=f"XT{k}")
        nc.vector.tensor_scalar_add(t[:], ps[:, :NPOS], bemb_pp[:, k:k + 1])
        XT_t[k] = t

    # ---------------- transformer layers ----------------
    for l in range(LYR):
        qlo, qhi = (0, NPOS) if l == 0 else (QLO2, NPOS)
        qw = qhi - qlo
        pos_chunks = ([(0, 0, 128), (1, 0, 128)] if l == 0
                      else [(1, 0, 96)])
        # (X-tile index, row offset within tile, nrows) for output positions
        if l == 1:
            # residual rows [32,128) of X_t[1] shifted to partition base 0
            ps = psw.tile([128, 1024], F32, tag="qk")
            nc.tensor.matmul(ps[:qw, :E], ident[:, 32:128], X_t[1][:],
                             start=True, stop=True)
            xr = work.tile([128, NPOS], F32, tag="xr")
            nc.vector.tensor_copy(xr[:qw], ps[:qw, :E])

        # ---- K projection -> KT channel-major bf16 [128, 256] x2
        KT = {}
        for Jt in range(2):
            ps = psw.tile([128, 1024], F32, tag="qk")
            for k in range(2):
                nc.tensor.matmul(
                    ps[:, :NPOS],
                    Wk_t[(l, k)][:, Jt * 128:(Jt + 1) * 128],
                    XT_t[k][:],
                    start=(k == 0), stop=(k == 1))
            t = work.tile([128, NPOS], BF16, tag=f"KT{Jt}")
            nc.vector.tensor_scalar_add(t[:], ps[:, :NPOS], bk_t[l][:, Jt:Jt + 1])
            KT[Jt] = t

        # ---- V projection -> V position-major bf16 [128, 256] x2
        V = {}
        for pc in range(2):
            ps = psw.tile([128, 1024], F32, tag="qk")
            seed_bias(ps[:, :E], brows[("bv", l)], 128, E)
            for k in range(2):
                nc.tensor.matmul(
                    ps[:, :E],
                    XT_t[k][:, pc * 128:(pc + 1) * 128],
                    Wv_t[(l, k)][:],
                    start=False, stop=(k == 1))
            t = work.tile([128, E], BF16, tag=f"V{pc}")
            nc.vector.tensor_copy(t[:], ps[:, :E])
            V[pc] = t

        # ---- Q projection -> Qs (decay-scaled) bf16, rotations r=0..3
        Qs = {}
        for h in range(2):
            ps = psw.tile([128, 1024], F32, tag="qk")
            for k in range(2):
                nc.tensor.matmul(
                    ps[:, :qw],
                    Wq_t[(l, k)][:, h * 128:(h + 1) * 128],
                    XT_t[k][:, qlo:qhi],
                    start=(k == 0), stop=(k == 1))
            tf = work.tile([128, NPOS], F32, tag="qtmp")
            nc.vector.tensor_scalar_add(tf[:, :qw], ps[:, :qw],
                                        bq_t[l][:, h:h + 1])
            t = work.tile([128, NPOS], BF16, tag=f"Qs0{h}")
            nc.vector.tensor_mul(t[:, :qw], tf[:, :qw], D_t[h][:, qlo:qhi])
            Qs[(0, h)] = t
        for r in range(1, 4):
            for h in range(2):
                ps = psw.tile([128, 1024], F32, tag="qk")
                nc.tensor.matmul(ps[:, :qw], Prot_t[r - 1][:],
                                 Qs[(0, h)][:, :qw], start=True, stop=True)
                t = work.tile([128, NPOS], BF16, tag=f"Qs{r}{h}")
                nc.vector.tensor_copy(t[:, :qw], ps[:, :qw])
                Qs[(r, h)] = t

        # ---- attention: ST -> exp -> A@V (+ row sums), accumulated in PSUM
        # Accumulators are zeroed by DVE memset; every matmul then uses
        # start=False (pure accumulate), so scheduler order within the
        # region doesn't matter.  skip_group_check silences the group
        # bookkeeping that this pattern sidesteps.
        OT_ps = psacc.tile([128, 2, NPOS], F32, tag="ot")   # [ch128, h, q]
        RS_ps = psacc.tile([128, 2, NPOS], F32, tag="rs")
        nc.vector.memset(OT_ps[:], 0.0)
        nc.vector.memset(RS_ps[:], 0.0)
        exp_i = 0
        for J in range(2):          # key c'-quad
            for pc in range(2):     # key position chunk
                ATl = {}
                for h in range(2):
                    for r in range(4):
                        # each matmul gets its own 2KB psum zero-region
                        # (slices padded to 512 f32): region-sharing with a
                        # split start/stop group crashes the device.  One
                        # 2-matmul tile per exp (not a psa/psb pair) so the
                        # 3-buffer psw ring pipelines deeper and the PE is
                        # less tightly coupled to the exp engines.
                        for half in range(2):
                            pst = psw.tile([128, 2, 512], F32, tag="qk")
                            for g in range(2):
                                i = 2 * half + g
                                nc.tensor.matmul(
                                    pst[:, g, :qw],
                                    KT[J][32 * i:32 * (i + 1),
                                          pc * 128:(pc + 1) * 128],
                                    Qs[(r, h)][32 * i:32 * (i + 1), :qw],
                                    start=True, stop=True,
                                    tile_position=(32 * i, 0))
                            at2 = atp.tile([128, 2, NPOS], BF16, tag="at")
                            if exp_i % 3 == 2:
                                ti = work.tile([128, 2, NPOS], I32, tag="ei")
                                nc.vector.tensor_scalar(ti[:, :, :qw],
                                                        pst[:, :, :qw],
                                                        SCH_A, SCH_B,
                                                        MULT, ADD)
                                nc.vector.tensor_copy(
                                    at2[:, :, :qw],
                                    ti[:, :, :qw].bitcast(F32))
                            else:
                                nc.scalar.activation(at2[:, :, :qw],
                                                     pst[:, :, :qw],
                                                     AF.Exp)
                            exp_i += 1
                            for g in range(2):
                                i = 2 * half + g
                                c = 4 * h + (i + r) % 4
                                ATl[(c, i)] = at2[:, g, :qw]
                for i in range(4):  # s-chunk (c' = 4J+i, pc)
                    cp = 4 * J + i
                    Vv = V[pc][:, 32 * cp:32 * (cp + 1)]   # [128, 32] bf16
                    for h in range(2):
                        for j in range(4):
                            nc.tensor.matmul(
                                OT_ps[32 * j:32 * (j + 1), h, :qw],
                                Vv, ATl[(4 * h + j, i)],
                                start=False, stop=False,
                                skip_group_check=True,
                                tile_position=(0, 32 * j))
                        for j in range(4):
                            nc.tensor.matmul(
                                RS_ps[32 * j:32 * j + 1, h, :qw],
                                ones_col[:], ATl[(4 * h + j, i)],
                                start=False, stop=False,
                                skip_group_check=True,
                                tile_position=(0, 32 * j))

        # ---- normalize: OT = OT * (1/RS) broadcast over the 32-row blocks
        # reciprocal once over the whole tile (unused partitions hold
        # garbage, never read), then K=1 matmuls broadcast row 32j over
        # its 32-row block.
        rcp = work.tile([128, 2, NPOS], F32, tag="rcp")
        nc.vector.reciprocal_approx_fast(rcp[:, :, :qw], RS_ps[:, :, :qw])
        RB_ps = psw.tile([128, 2, 512], F32, tag="qk")
        nc.vector.memset(RB_ps[:, :, :qw], 0.0)
        for h in range(2):
            for j in range(4):
                nc.tensor.matmul(
                    RB_ps[32 * j:32 * (j + 1), h, :qw],
                    ones32[32 * j:32 * j + 1, :32],
                    rcp[32 * j:32 * j + 1, h, :qw],
                    start=False, stop=False, skip_group_check=True,
                    tile_position=(32 * j, 32 * j))
        rb_sb = work.tile([128, 2, NPOS], F32, tag="rb_sb")
        nc.vector.tensor_copy(rb_sb[:, :, :qw], RB_ps[:, :, :qw])
        ot_t = work.tile([128, 2, NPOS], WDT, tag="OTn")
        nc.vector.tensor_tensor(ot_t[:, :, :qw], OT_ps[:, :, :qw],
                                rb_sb[:, :, :qw], MULT)
        OT_sb = {h: ot_t[:, h, :qw] for h in range(2)}

        # ---- O @ Wo + bo + residual -> LN1 -> xa
        xa = {}
        for ci, (xi, ro, nr) in enumerate(pos_chunks):
            ps = psw.tile([128, 1024], F32, tag="qk")
            seed_bias(ps[:nr, :E], brows[("bo", l)], nr, E)
            for h in range(2):
                nc.tensor.matmul(
                    ps[:nr, :E],
                    OT_sb[h][:, ci * 128:ci * 128 + nr],
                    Wo_t[(l, h)][:],
                    start=False, stop=(h == 1))
            res = work.tile([128, NPOS], F32, tag=f"res{ci}")
            rsrc = X_t[xi][ro:ro + nr, :] if l == 0 else xr[:nr, :]
            nc.vector.tensor_add(res[:nr], ps[:nr, :E], rsrc)
            t = work.tile([128, NPOS], F32, tag=f"xa{ci}")
            layernorm(res[:nr], nr, ln_b[("ln1g", l)], ln_b[("ln1b", l)],
                      t[:nr])
            xa[ci] = t

        # ---- transpose xa -> xaT channel-major
        xaT = {}
        for k in range(2):
            t = work.tile([128, NPOS], WDT, tag=f"xaT{k}")
            for ci, (_, _, nr) in enumerate(pos_chunks):
                ps = psw.tile([128, 1024], F32, tag="qk")
                nc.tensor.transpose(ps[:, :nr],
                                    xa[ci][:nr, k * 128:(k + 1) * 128],
                                    ident[:nr, :nr])
                nc.vector.tensor_copy(t[:, ci * 128:ci * 128 + nr],
                                      ps[:, :nr])
            xaT[k] = t

        # ---- FFN: H1T = relu(W1.T x + b1) channel-major bf16 [128, qw] x8
        H1T = {}
        for dk in range(8):
            ps = psw.tile([128, 1024], F32, tag="qk")
            for k in range(2):
                nc.tensor.matmul(
                    ps[:, :qw],
                    W1_t[(l, k)][:, dk * 128:(dk + 1) * 128],
                    xaT[k][:, :qw],
                    start=(k == 0), stop=(k == 1))
            t = work.tile([128, NPOS], BF16, tag=f"H1T{dk}")
            nc.vector.tensor_scalar(t[:, :qw], ps[:, :qw],
                                    b1_t[l][:, dk:dk + 1], 0.0, ADD, MAX)
            H1T[dk] = t

        # ---- FF = relu(H1 @ W2 + b2); X_next = LN2(xa + FF)
        newX = {}
        for ci, (_, _, nr) in enumerate(pos_chunks):
            ps = psw.tile([128, 1024], F32, tag="qk")
            seed_bias(ps[:nr, :E], brows[("b2", l)], nr, E)
            for dk in range(8):
                nc.tensor.matmul(
                    ps[:nr, :E],
                    H1T[dk][:, ci * 128:ci * 128 + nr],
                    W2_t[(l, dk)][:],
                    start=False, stop=(dk == 7))
            t = work.tile([128, NPOS], F32, tag=f"ff{ci}")
            nc.vector.tensor_scalar_max(t[:nr], ps[:nr, :E], 0.0)
            res2 = work.tile([128, NPOS], F32, tag=f"res2{ci}")
            nc.vector.tensor_add(res2[:nr], t[:nr], xa[ci][:nr])
            xn = const.tile([128, NPOS], F32, tag=f"Xn{l}{ci}")
            layernorm(res2[:nr], nr, ln_b[("ln2g", l)], ln_b[("ln2b", l)],
                      xn[:nr])
            newX[ci] = xn

        if l == 0:
            X_t = {0: newX[0], 1: newX[1]}
            XT_t = {}
            for k in range(2):
                t = const.tile([128, NPOS], WDT, tag=f"X1T{k}")
                for ci in range(2):
                    ps = psw.tile([128, 1024], F32, tag="qk")
                    nc.tensor.transpose(ps[:, :128],
                                        newX[ci][:, k * 128:(k + 1) * 128],
                                        ident[:])
                    nc.vector.tensor_copy(t[:, ci * 128:(ci + 1) * 128],
                                          ps[:, :128])
                XT_t[k] = t
        else:
            X2 = newX[0]  # [96, 256]: positions 160..255

    # ---------------- final LN + projection (row output) ----------------
    xf = work.tile([128, NPOS], F32, tag="xf")
    layernorm(X2[:PRED], PRED, ln_b["lnfg"], ln_b["lnfb"], xf[:PRED])
    xfT = {}
    for k in range(2):
        ps = psw.tile([128, 1024], F32, tag="qk")
        nc.tensor.transpose(ps[:, :PRED], xf[:PRED, k * 128:(k + 1) * 128],
                            ident[:PRED, :PRED])
        t = work.tile([128, 128], F32, tag=f"xfT{k}")
        nc.vector.tensor_copy(t[:, :PRED], ps[:, :PRED])
        xfT[k] = t
    ps = psw.tile([128, 1024], F32, tag="qk")
    nc.tensor.matmul(ps[0:1, :PRED], bprow[0:1, 0:1],
                     ones_row[0:1, :PRED], start=True, stop=False)
    for k in range(2):
        nc.tensor.matmul(ps[0:1, :PRED], Wp_sb[:, k:k + 1],
                         xfT[k][:, :PRED],
                         start=False, stop=(k == 1))
    ot = work.tile([1, PRED], F32, tag="outsb")
    nc.vector.tensor_copy(ot[:], ps[0:1, :PRED])
    nc.sync.dma_start(out=outs["out"][:], in_=ot[:])


# ======================= host side =======================

def _rot_matrices():
    """P_r[k, m] = 1 iff k = 32*((m//32 + r) % 4) + m % 32, r = 1..3."""
    import ml_dtypes
    mats = np.zeros((3, 128, 128), np.float32)
    for r in range(1, 4):
        for m in range(128):
            mats[r - 1, 32 * ((m // 32 + r) % 4) + m % 32, m] = 1.0
    return mats.astype(ml_dtypes.bfloat16)


def _make_in_maps(inputs):
    import ml_dtypes
    f = np.float32
    bh = ml_dtypes.bfloat16
    x_enc = np.asarray(inputs["x_enc"], f)
    td = np.asarray(inputs["time_diffs"], f)

    def g(nm):
        return np.asarray(inputs[nm], f)

    # ---- PF1: pre-broadcast LN tiles
    pf1 = np.zeros((128, CF1), f)
    ln_list = [g("ln1_g")[0], g("ln1_b")[0], g("ln2_g")[0], g("ln2_b")[0],
               g("ln1_g")[1], g("ln1_b")[1], g("ln2_g")[1], g("ln2_b")[1],
               g("lnf_g"), g("lnf_b")]
    for i, v in enumerate(ln_list):
        pf1[:, 256 * i:256 * (i + 1)] = v[None, :]

    # ---- PB0: Wq/Wk/Wv + Prot + bias rows
    pb0 = np.zeros((128, CB0), bh)
    for l in range(LYR):
        for k in range(2):
            i = 2 * l + k
            pb0[:, WQ_O + 256 * i:WQ_O + 256 * (i + 1)] = \
                g("Wq")[l, 128 * k:128 * (k + 1), :].astype(bh)
            pb0[:, WK_O + 256 * i:WK_O + 256 * (i + 1)] = \
                g("Wk")[l, 128 * k:128 * (k + 1), :].astype(bh)
            pb0[:, WV_O + 256 * i:WV_O + 256 * (i + 1)] = \
                g("Wv")[l, 128 * k:128 * (k + 1), :].astype(bh)
    prot = _rot_matrices()
    for r in range(3):
        pb0[:, PROT_O + 128 * r:PROT_O + 128 * (r + 1)] = prot[r]
    for i, (nm, l) in enumerate([("bv", 0), ("bv", 1), ("bo", 0),
                                 ("bo", 1), ("b2", 0), ("b2", 1)]):
        pb0[0, BROW_O + 256 * i:BROW_O + 256 * (i + 1)] = \
            g(nm)[l].astype(bh)

    # ---- PB1: Wo + W1 + W2
    pb1 = np.zeros((128, CB1), bh)
    for l in range(LYR):
        for h in range(2):
            i = 2 * l + h
            pb1[:, WO_O + 256 * i:WO_O + 256 * (i + 1)] = \
                g("Wo")[l, 128 * h:128 * (h + 1), :].astype(bh)
        for k in range(2):
            i = 2 * l + k
            pb1[:, W1_O + 1024 * i:W1_O + 1024 * (i + 1)] = \
                g("W1")[l, 128 * k:128 * (k + 1), :].astype(bh)
        for dk in range(8):
            i = 8 * l + dk
            pb1[:, W2_O + 256 * i:W2_O + 256 * (i + 1)] = \
                g("W2")[l, 128 * dk:128 * (dk + 1), :].astype(bh)

    # ---- PF0 (per-core: contains xT and td)
    pf0_base = np.zeros((128, CF0), f)
    pf0_base[0:7, WEMB_O:WEMB_O + 256] = g("W_emb")  # [7, 256]
    pf0_base[:, BEMBPP_O:BEMBPP_O + 2] = g("b_emb").reshape(2, 128).T
    pf0_base[0, BEMBR_O:BEMBR_O + 256] = g("b_emb")
    for l in range(LYR):
        pf0_base[:, BQ_O + 2 * l:BQ_O + 2 * l + 2] = \
            g("bq")[l].reshape(2, 128).T
        pf0_base[:, BK_O + 2 * l:BK_O + 2 * l + 2] = \
            g("bk")[l].reshape(2, 128).T
        pf0_base[:, B1_O + 8 * l:B1_O + 8 * l + 8] = \
            g("b1")[l].reshape(8, 128).T
    pf0_base[:, WP_O:WP_O + 2] = g("W_proj")[:, 0].reshape(2, 128).T
    pf0_base[0, BPROJ_O] = g("b_proj")[0]

    pb0 = np.ascontiguousarray(pb0)
    pb1 = np.ascontiguousarray(pb1)
    pf1 = np.ascontiguousarray(pf1)
    maps = []
    for b in range(B):
        pf0 = pf0_base.copy()
        # td8row[0, c*256 + q] = td[b, q*8 + c]  (head-view c-major order)
        pf0[0, TD8R:TD8R + L] = td[b].reshape(256, 8).T.reshape(-1)
        pf0[0:7, XT_O:XT_O + 256] = x_enc[b, P0:P0 + NPOS, :].T
        maps.append({"PF0": np.ascontiguousarray(pf0), "PB0": pb0,
                     "PF1": pf1, "PB1": pb1})
    return maps


OUT_SHAPE = (1, PRED)


def _assemble(results, expected_like=None):
    return np.stack(
        [list(results[b].values())[0].reshape(PRED) for b in range(B)]
    ).astype(np.float32)


def _run(in_maps, check_with_sim=False, check_with_hw=True, **kw):
    from concourse.bass_test_utils import run_kernel

    n = len(in_maps)
    out_like = {"out": np.zeros(OUT_SHAPE, np.float32)}
    res = run_kernel(
        lambda tc, outs, ins: chaos_kernel(tc, outs, ins),
        None,
        in_maps if n > 1 else in_maps[0],
        output_like=[out_like] * n if n > 1 else out_like,
        bass_type=tile.TileContext,
        num_cores=n,
        check_with_sim=check_with_sim,
        check_with_hw=check_with_hw,
        trace_sim=False,
        **kw,
    )
    return res


def kernel(**inputs):
    in_maps = _make_in_maps(inputs)
    res = _run(in_maps)
    return _assemble(res.results)
